# revision 1
# baseline (speedup 1.0000x reference)
"""BiLSTM (2-layer, masked/ragged) Trainium2 kernel.

Sharding: 8 cores = 2 directions x 4 batch shards (16 each). Backward
direction cores receive time-reversed inputs from the host, so the device
program is direction-agnostic SPMD. Layer-0 direction outputs are swapped
between fwd/bwd partner cores with an 8-core AllGather of time-reversed
copies; each core then computes layer-1 input projections from its own +
partner halves and runs the layer-1 scan.

All matmuls in bf16 (weights stationary, gates on PSUM partitions, batch on
the free dim), cell state and elementwise chain in fp32.
"""

import os
import numpy as np
import ml_dtypes

import concourse.bass as bass
import concourse.bacc as bacc
import concourse.mybir as mybir
import concourse.tile as tile
from concourse import bass_utils

bf16 = ml_dtypes.bfloat16
f32 = mybir.dt.float32
bf = mybir.dt.bfloat16

T, B, D, H = 512, 64, 512, 512
NCORES = 8
BS = B // 4  # 16, batch shard per core
G = 4 * H  # 2048 gates
GT = G // 128  # 16 gate tiles
KH = H // 128  # 4 k-chunks for hidden contraction
TC = 32  # timesteps per phase chunk
NC_CHUNKS = T // TC  # 16

_compiled = {}

# gate-column permutation: store gate tile g (type q=g//4 in i,f,g,o; hidden
# tile j=g%4) at column block perm(g) so the layout is [i,f,o,g] and one
# sigmoid covers i,f,o contiguously.
_PERM = [(q, j) for q in range(4) for j in range(4)]
def _perm(g):
    q, j = g // 4, g % 4
    return {0: j, 1: 4 + j, 2: 12 + j, 3: 8 + j}[q]


def _build(t_steps=T, dbg=False):
    nc = bacc.Bacc("TRN2", target_bir_lowering=False, debug=False,
                   num_devices=NCORES)
    nchunks = t_steps // TC

    # ---- per-core external inputs ----
    xT = nc.dram_tensor("xT", (D, t_steps, BS), bf, kind="ExternalInput")
    maskd = nc.dram_tensor("maskd", (t_steps, KH * BS), bf, kind="ExternalInput")
    whh0T = nc.dram_tensor("whh0T", (KH, 128, G), bf, kind="ExternalInput")
    wih0T = nc.dram_tensor("wih0T", (KH, 128, G), bf, kind="ExternalInput")
    whh1T = nc.dram_tensor("whh1T", (KH, 128, G), bf, kind="ExternalInput")
    wih1oT = nc.dram_tensor("wih1oT", (KH, 128, G), bf, kind="ExternalInput")
    wih1pT = nc.dram_tensor("wih1pT", (KH, 128, G), bf, kind="ExternalInput")
    b0c = nc.dram_tensor("b0c", (GT, 128), f32, kind="ExternalInput")
    b1c = nc.dram_tensor("b1c", (GT, 128), f32, kind="ExternalInput")
    y1 = nc.dram_tensor("y1", (t_steps, 128, KH * BS), f32, kind="ExternalOutput")
    if dbg:
        xwb0o = nc.dram_tensor("xwb0o", (t_steps, 128, GT * BS), bf, kind="ExternalOutput")
        xwb1o = nc.dram_tensor("xwb1o", (t_steps, 128, GT * BS), bf, kind="ExternalOutput")
        y0o = nc.dram_tensor("y0o", (t_steps, 128, KH * BS), bf, kind="ExternalOutput")
        ago = nc.dram_tensor("ago", (NCORES * t_steps, 128, KH * BS), bf, kind="ExternalOutput")

    with tile.TileContext(nc) as tc:
        with (
            tc.tile_pool(name="wpool", bufs=1) as wpool,
            tc.tile_pool(name="xpool", bufs=3) as xpool,
            tc.tile_pool(name="gpool", bufs=3) as gpool,
            tc.tile_pool(name="spool", bufs=3) as spool,
            tc.tile_pool(name="state", bufs=1) as state,
            tc.tile_pool(name="psA", bufs=2, space="PSUM") as psA,
            tc.tile_pool(name="psS", bufs=2, space="PSUM") as psS,
            tc.tile_pool(name="dram", bufs=1, space="DRAM") as dram,
        ):
            # ---- internal DRAM ----
            xwb0 = dram.tile([t_steps, 128, GT * BS], bf)
            xwb1 = dram.tile([t_steps, 128, GT * BS], bf)
            y0 = dram.tile([t_steps, 128, KH * BS], bf)
            y0x = dram.tile([t_steps, 128, KH * BS], bf)
            ag = dram.tile([NCORES * t_steps, 128, KH * BS], bf)

            # ---- resident weights ----
            def load_w(name, src):
                t = wpool.tile([128, KH * G], bf, tag=name)
                for k in range(KH):
                    nc.sync.dma_start(t[:, k * G:(k + 1) * G], src.ap()[k])
                return t

            whh0_sb = load_w("whh0", whh0T)
            wih0_sb = load_w("wih0", wih0T)
            whh1_sb = load_w("whh1", whh1T)
            wih1o_sb = load_w("wih1o", wih1oT)
            wih1p_sb = load_w("wih1p", wih1pT)
            bias_sb = wpool.tile([128, 2 * GT], f32, tag="bias")
            nc.sync.dma_start(bias_sb[:, 0:GT], b0c.ap().transpose([1, 0]))
            nc.sync.dma_start(bias_sb[:, GT:2 * GT], b1c.ap().transpose([1, 0]))

            # ---- phase A / D: input projections -> xwb dram ----
            def proj(dst, w_sbs, srcs, bias_col, dbg_dst=None):
                """dst[t,p,g*BS+b] = sum_j srcs[j] @ w_sbs-chunks + bias."""
                nk = len(w_sbs) * KH
                for ncnk in range(nchunks):
                    t0 = ncnk * TC
                    rhs = xpool.tile([128, nk, TC, BS], bf, tag="projx")
                    ji = 0
                    for w_sb, src in zip(w_sbs, srcs):
                        for k in range(KH):
                            nc.sync.dma_start(rhs[:, ji], src(k, t0))
                            ji += 1
                    for g in range(GT):
                        ps = psA.tile([128, TC * BS], f32, tag="psA")
                        ji = 0
                        for w_sb in w_sbs:
                            for k in range(KH):
                                nc.tensor.matmul(
                                    ps[:],
                                    w_sb[:, k * G + g * 128: k * G + (g + 1) * 128],
                                    rhs[:, ji],
                                    start=(ji == 0),
                                    stop=(ji == nk - 1),
                                )
                                ji += 1
                        g_sb = gpool.tile([128, TC * BS], bf, tag="projg")
                        nc.scalar.activation(
                            g_sb[:], ps[:], mybir.ActivationFunctionType.Identity,
                            bias=bias_sb[:, bias_col + g: bias_col + g + 1],
                        )
                        P = _perm(g)
                        nc.sync.dma_start(
                            dst[t0:t0 + TC, :, P * BS:(P + 1) * BS].transpose([1, 0, 2]),
                            g_sb[:].rearrange("p (t b) -> p t b", t=TC),
                        )
                        if dbg_dst is not None:
                            nc.sync.dma_start(
                                dbg_dst.ap()[t0:t0 + TC, :, g * BS:(g + 1) * BS].transpose([1, 0, 2]),
                                g_sb[:].rearrange("p (t b) -> p t b", t=TC),
                            )

            proj(
                xwb0, [wih0_sb],
                [lambda k, t0: xT.ap()[k * 128:(k + 1) * 128, t0:t0 + TC, :]],
                bias_col=0,
                dbg_dst=xwb0o if dbg else None,
            )

            # ---- scan helper ----
            def scan(xwb, whh_sb, y_dst, yx_dst, out_f32, dbg_dst=None):
                h2f = state.tile([128, KH * BS], f32, tag="h2f")
                cst = state.tile([128, KH * BS], f32, tag="cst")
                nc.gpsimd.memset(h2f[:], 0.0)
                nc.gpsimd.memset(cst[:], 0.0)
                for t in range(t_steps):
                    m_sb = spool.tile([128, KH * BS], bf, tag="m")
                    nc.sync.dma_start(
                        m_sb[:], maskd.ap()[t:t + 1, :].broadcast_to([128, KH * BS]))
                    xw_sb = spool.tile([128, GT * BS], bf, tag="xw")
                    nc.sync.dma_start(xw_sb[:], xwb[t])
                    h_in = spool.tile([128, KH * BS], bf, tag="hin")
                    nc.vector.tensor_mul(h_in[:], h2f[:], m_sb[:])
                    nc.vector.tensor_mul(cst[:], cst[:], m_sb[:])
                    ps = psS.tile([128, GT * BS], f32, tag="psS")
                    for g in range(GT):
                        P = _perm(g)
                        for k in range(KH):
                            nc.tensor.matmul(
                                ps[:, P * BS:(P + 1) * BS],
                                whh_sb[:, k * G + g * 128: k * G + (g + 1) * 128],
                                h_in[:, k * BS:(k + 1) * BS],
                                start=(k == 0),
                                stop=(k == KH - 1),
                            )
                    nc.vector.tensor_add(ps[:], ps[:], xw_sb[:])
                    # permuted gate cols: [0:H') i | [H':2H') f | [2H':3H') o
                    # | [3H':4H') g-candidate  (H'=KH*BS)
                    HB = KH * BS
                    sif = spool.tile([128, 3 * HB], f32, tag="sif")
                    nc.scalar.activation(
                        sif[:], ps[:, 0:3 * HB], mybir.ActivationFunctionType.Sigmoid)
                    tng = spool.tile([128, HB], f32, tag="tng")
                    nc.scalar.activation(
                        tng[:], ps[:, 3 * HB:4 * HB], mybir.ActivationFunctionType.Tanh)
                    ig = spool.tile([128, HB], f32, tag="ig")
                    nc.vector.tensor_mul(ig[:], sif[:, 0:HB], tng[:])
                    fc = spool.tile([128, HB], f32, tag="fc")
                    nc.vector.tensor_mul(fc[:], sif[:, HB:2 * HB], cst[:])
                    nc.vector.tensor_add(cst[:], fc[:], ig[:])
                    tc2 = spool.tile([128, HB], f32, tag="tc2")
                    nc.scalar.activation(
                        tc2[:], cst[:], mybir.ActivationFunctionType.Tanh)
                    nc.vector.tensor_mul(h2f[:], sif[:, 2 * HB:3 * HB], tc2[:])
                    y_sb = spool.tile([128, HB], f32 if out_f32 else bf, tag="y")
                    nc.vector.tensor_mul(y_sb[:], h2f[:], m_sb[:])
                    nc.sync.dma_start(y_dst[t], y_sb[:])
                    if yx_dst is not None:
                        nc.sync.dma_start(yx_dst[t_steps - 1 - t], y_sb[:])
                    if dbg_dst is not None:
                        nc.sync.dma_start(dbg_dst.ap()[t], y_sb[:])

            scan(xwb0, whh0_sb, y0, y0x, out_f32=False,
                 dbg_dst=y0o if dbg else None)

            # ---- exchange ----
            nc.gpsimd.collective_compute(
                "AllGather", mybir.AluOpType.bypass,
                ins=[y0x.opt()], outs=[ag.opt()],
                replica_groups=[list(range(NCORES))],
            )
            partner_row = nc.snap(((nc.partition_id() + 4) % 8) * t_steps)

            if dbg:
                nc.sync.dma_start(ago.ap()[:], ag[:])

            proj(
                xwb1, [wih1o_sb, wih1p_sb],
                [
                    lambda k, t0: y0[t0:t0 + TC, :, k * BS:(k + 1) * BS].transpose([1, 0, 2]),
                    lambda k, t0: ag[bass.ds(partner_row + t0, TC), :, k * BS:(k + 1) * BS].transpose([1, 0, 2]),
                ],
                bias_col=GT,
                dbg_dst=xwb1o if dbg else None,
            )

            scan(xwb1, whh1_sb, y1.ap(), None, out_f32=True)

    nc.compile()
    return nc


def _prep_inputs(x, lengths, weights, t_steps=T):
    """Build the 8 per-core input maps."""
    active = (np.arange(T)[:, None] < np.asarray(lengths)[None, :]).astype(np.float32)
    in_maps = []
    for c in range(NCORES):
        d, s = c // 4, c % 4
        bsl = slice(s * BS, (s + 1) * BS)
        pre = "f" if d == 0 else "b"
        xs = np.asarray(x[:, bsl, :], np.float32)
        am = active[:, bsl]
        if d == 1:
            xs = xs[::-1]
            am = am[::-1]
        xs = xs[:t_steps]
        am = am[:t_steps]

        W_ih0 = np.asarray(weights[f"{pre}W_ih0"], np.float32)
        W_hh0 = np.asarray(weights[f"{pre}W_hh0"], np.float32)
        W_ih1 = np.asarray(weights[f"{pre}W_ih1"], np.float32)
        W_hh1 = np.asarray(weights[f"{pre}W_hh1"], np.float32)
        b0 = np.asarray(weights[f"{pre}b0"], np.float32)
        b1 = np.asarray(weights[f"{pre}b1"], np.float32)
        own = W_ih1[:, :512] if d == 0 else W_ih1[:, 512:]
        par = W_ih1[:, 512:] if d == 0 else W_ih1[:, :512]

        in_maps.append({
            "xT": np.ascontiguousarray(xs.transpose(2, 0, 1)).astype(bf16),
            "maskd": np.ascontiguousarray(np.tile(am, (1, KH))).astype(bf16),
            "whh0T": np.ascontiguousarray(W_hh0.T.reshape(KH, 128, G)).astype(bf16),
            "wih0T": np.ascontiguousarray(W_ih0.T.reshape(KH, 128, G)).astype(bf16),
            "whh1T": np.ascontiguousarray(W_hh1.T.reshape(KH, 128, G)).astype(bf16),
            "wih1oT": np.ascontiguousarray(own.T.reshape(KH, 128, G)).astype(bf16),
            "wih1pT": np.ascontiguousarray(par.T.reshape(KH, 128, G)).astype(bf16),
            "b0c": np.ascontiguousarray(b0.reshape(GT, 128)).astype(np.float32),
            "b1c": np.ascontiguousarray(b1.reshape(GT, 128)).astype(np.float32),
        })
    return in_maps


def _assemble(results, t_steps=T):
    out = np.zeros((t_steps, B, 2 * H), np.float32)
    for c in range(NCORES):
        d, s = c // 4, c % 4
        arr = results[c]["y1"].reshape(t_steps, 128, KH, BS)
        if d == 1:
            arr = arr[::-1]
        # [t, p, j, b] -> [t, b, j*128+p]
        blk = arr.transpose(0, 3, 2, 1).reshape(t_steps, BS, H)
        out[:, s * BS:(s + 1) * BS, d * H:(d + 1) * H] = blk
    return out


def kernel(x, lengths, fW_ih0, fW_hh0, fb0, bW_ih0, bW_hh0, bb0,
           fW_ih1, fW_hh1, fb1, bW_ih1, bW_hh1, bb1, _t_steps=T,
           _want_trace=False, _dbg=False):
    weights = dict(fW_ih0=fW_ih0, fW_hh0=fW_hh0, fb0=fb0,
                   bW_ih0=bW_ih0, bW_hh0=bW_hh0, bb0=bb0,
                   fW_ih1=fW_ih1, fW_hh1=fW_hh1, fb1=fb1,
                   bW_ih1=bW_ih1, bW_hh1=bW_hh1, bb1=bb1)
    key = (_t_steps, _dbg)
    if key not in _compiled:
        _compiled[key] = _build(_t_steps, dbg=_dbg)
    nc = _compiled[key]
    in_maps = _prep_inputs(x, lengths, weights, _t_steps)
    res = bass_utils.run_bass_kernel_spmd(
        nc, in_maps, core_ids=list(range(NCORES)), trace=_want_trace)
    out = _assemble(res.results, _t_steps)
    if _want_trace or _dbg:
        kernel.last_results = res
    return out



# revision 8
# speedup vs baseline: 1.9506x; 1.9506x over previous
"""BiLSTM (2-layer, masked/ragged) Trainium2 kernel, v2.

Sharding: 8 cores = 2 directions x 4 batch shards (BS=16 each). Backward
cores receive time-reversed inputs from the host, so the device program is
direction-agnostic SPMD. Layer-0 outputs are exchanged between fwd/bwd
partner cores with an 8-core AllGather of time-reversed copies.

v2 structure (vs the v1 baseline):
- Input projections write gate pre-activations (bias folded) straight into
  the scan's SBUF chunk tiles -- no DRAM bounce, no strided DMA.
- Projection work for chunk c+1 is interleaved into the scan of chunk c,
  filling PE bubbles left by the serial cell chain.
- Gates grouped by type (i,f,g,o) into four separate PSUM banks; each
  type's xw-add + activation runs while the next type's matmuls stream.
- Per-step state masking and the y outputs ride on the Pool engine; the
  serial chain is MMs -> add -> ACT -> (mul/add) -> tanh -> mul.
- xw and masks live in per-chunk SBUF tiles (double-buffered); the only
  per-step DMA is none -- stores are chunk-granular and contiguous.
"""

import numpy as np
import ml_dtypes

import concourse.bass as bass
import concourse.bacc as bacc
import concourse.mybir as mybir
import concourse.tile as tile
from concourse import bass_utils

bf16 = ml_dtypes.bfloat16
f32 = mybir.dt.float32
bf = mybir.dt.bfloat16

T, B, D, H = 512, 64, 512, 512
NCORES = 8
BS = B // 4          # 16 batch per core
G = 4 * H            # 2048 gates
GT = G // 128        # 16 gate tiles
KH = H // 128        # 4 contraction chunks for H
TC = 32              # timesteps per chunk
FD = TC * BS         # 512, proj matmul free dim

Tanh = mybir.ActivationFunctionType.Tanh
Sigmoid = mybir.ActivationFunctionType.Sigmoid
Identity = mybir.ActivationFunctionType.Identity

# gate-type processing order: g (candidates) first so its tanh overlaps the
# i/f/o matmuls; o last feeds the tail.
QORDER = (2, 0, 1, 3)

_compiled = {}


def _build(t_steps=T):
    assert t_steps % TC == 0
    NC = t_steps // TC
    nc = bacc.Bacc("TRN2", target_bir_lowering=False, debug=False,
                   num_devices=NCORES)

    # ---- per-core external inputs ----
    xT = nc.dram_tensor("xT", (D, t_steps, BS), bf, kind="ExternalInput")
    maskd = nc.dram_tensor("maskd", (NC, (TC + 1) * KH * BS), bf,
                           kind="ExternalInput")
    whh0T = nc.dram_tensor("whh0T", (KH, 128, G), bf, kind="ExternalInput")
    wih0T = nc.dram_tensor("wih0T", (KH, 128, G), bf, kind="ExternalInput")
    whh1T = nc.dram_tensor("whh1T", (KH, 128, G), bf, kind="ExternalInput")
    wih1oT = nc.dram_tensor("wih1oT", (KH, 128, G), bf, kind="ExternalInput")
    wih1pT = nc.dram_tensor("wih1pT", (KH, 128, G), bf, kind="ExternalInput")
    ident = nc.dram_tensor("ident", (128, 128), bf, kind="ExternalInput")
    b0c = nc.dram_tensor("b0c", (GT, 128), f32, kind="ExternalInput")
    b1c = nc.dram_tensor("b1c", (GT, 128), f32, kind="ExternalInput")
    y1 = nc.dram_tensor("y1", (KH, NC, 128, TC, BS), bf,
                        kind="ExternalOutput")

    with tile.TileContext(nc) as tc:
        with (
            tc.tile_pool(name="wpool", bufs=1) as wpool,
            tc.tile_pool(name="xwpool", bufs=2) as xwpool,
            tc.tile_pool(name="mpool", bufs=2) as mpool,
            tc.tile_pool(name="rhspool", bufs=2) as rhspool,
            tc.tile_pool(name="ypool", bufs=2) as ypool,
            tc.tile_pool(name="spool", bufs=3) as spool,
            tc.tile_pool(name="state", bufs=1) as state,
            tc.tile_pool(name="psS", bufs=1, space="PSUM") as psS,
            tc.tile_pool(name="psA", bufs=2, space="PSUM") as psA,
            tc.tile_pool(name="dram", bufs=1, space="DRAM") as dram,
        ):
            # ---- internal DRAM ----
            y0 = dram.tile([KH, NC, 128, TC, BS], bf)
            y0x = dram.tile([KH * NC, 128, TC, BS], bf)
            ag = dram.tile([NCORES * KH * NC, 128, TC, BS], bf)

            # ---- resident weights ----
            def load_w(name, src):
                t = wpool.tile([128, KH * G], bf, tag=name)
                for k in range(KH):
                    nc.sync.dma_start(t[:, k * G:(k + 1) * G], src.ap()[k])
                return t

            whh0_sb = load_w("whh0", whh0T)
            wih0_sb = load_w("wih0", wih0T)
            whh1_sb = load_w("whh1", whh1T)
            wih1o_sb = load_w("wih1o", wih1oT)
            wih1p_sb = load_w("wih1p", wih1pT)
            ident_sb = wpool.tile([128, 128], bf, tag="ident")
            nc.sync.dma_start(ident_sb[:], ident.ap())
            bias_sb = wpool.tile([128, 2 * GT], f32, tag="bias")
            nc.sync.dma_start(bias_sb[:, 0:GT], b0c.ap().transpose([1, 0]))
            nc.sync.dma_start(bias_sb[:, GT:2 * GT],
                              b1c.ap().transpose([1, 0]))

            partner_base = nc.snap(
                ((nc.partition_id() + 4) % NCORES) * (KH * NC))

            # ---------- projection codegen ----------
            def stage_rhs_l0(c):
                """Stage x chunk c into SBUF: [128, KH, TC, BS]."""
                rhs = rhspool.tile([128, KH, TC, BS], bf, tag="rhs0")
                t0 = c * TC
                for k in range(KH):
                    nc.sync.dma_start(
                        rhs[:, k],
                        xT.ap()[k * 128:(k + 1) * 128, t0:t0 + TC, :])
                return rhs

            def stage_rhs_l1(c):
                orhs = rhspool.tile([128, KH, TC, BS], bf, tag="rhs1o")
                nc.sync.dma_start(orhs[:],
                                  y0[:, c].transpose([1, 0, 2, 3]))
                prhs = rhspool.tile([128, KH, TC, BS], bf, tag="rhs1p")
                for k in range(KH):
                    nc.sync.dma_start(
                        prhs[:, k:k + 1],
                        ag[bass.ds(partner_base + k * NC + c, 1)]
                        .transpose([1, 0, 2, 3]))
                return orhs, prhs

            # xw storage block for natural gate tile g=(q*KH+j):
            # i->0..3, f->4..7, o->8..11, g(cand)->12..15
            def _sb_block(g):
                q, j = g // KH, g % KH
                return {0: 0, 1: KH, 2: 3 * KH, 3: 2 * KH}[q] + j

            def proj_tile(g, w_rhs, xw_dst, bias_col):
                """One gate tile's projection: accumulate over (w, rhs, k)
                pairs, then bias + cast into xw_dst[:, _sb_block(g)]."""
                ps = psA.tile([128, TC * BS], f32, tag="psA")
                nk = len(w_rhs) * KH
                ji = 0
                for w_sb, rhs in w_rhs:
                    for k in range(KH):
                        nc.tensor.matmul(
                            ps[:],
                            w_sb[:, k * G + g * 128: k * G + (g + 1) * 128],
                            rhs[:, k],
                            start=(ji == 0),
                            stop=(ji == nk - 1),
                        )
                        ji += 1
                nc.scalar.activation(
                    xw_dst[:, _sb_block(g)], ps[:], Identity,
                    bias=bias_sb[:, bias_col + g: bias_col + g + 1],
                )

            def new_xw_tile():
                return xwpool.tile([128, GT, TC * BS], bf, tag="xw", name="xw")

            def new_mask_tile(c):
                m = mpool.tile([128, (TC + 1) * KH * BS], bf, tag="m", name="m")
                nc.sync.dma_start(
                    m[:],
                    maskd.ap()[c:c + 1, :].broadcast_to(
                        [128, (TC + 1) * KH * BS]))
                return m

            # ---------- scan codegen ----------
            def scan(whh_sb, xw_tiles, m_tiles, layer):
                """xw_tiles/m_tiles: lists indexed by chunk; entries for
                c >= 1 may be filled lazily by the hook."""
                h_in = state.tile([128, KH, BS], bf, tag=f"hin{layer}")
                cst = state.tile([128, KH, BS], f32, tag=f"cst{layer}")
                nc.gpsimd.memset(h_in[:], 0.0)
                nc.gpsimd.memset(cst[:], 0.0)
                ps_g = psS.tile([128, KH, BS], f32, tag="psg", name="psg")
                ps_if = psS.tile([128, 2 * KH, BS], f32, tag="psif",
                                 name="psif")
                ps_o = psS.tile([128, KH, BS], f32, tag="pso", name="pso")

                for c in range(NC):
                    xw = xw_tiles[c]
                    m_sb = m_tiles[c]
                    ych = ypool.tile([128, KH, TC, BS], bf, tag="ych")
                    if layer == 0:
                        ychr = ypool.tile([128, KH, TC, BS], bf, tag="ychr")

                    # next chunk's mask DMA (prefetch)
                    if c + 1 < NC and m_tiles[c + 1] is None:
                        m_tiles[c + 1] = new_mask_tile(c + 1)

                    # interleave hooks: build proj for chunk c+1
                    hooks = _hooks(layer, c) if c + 1 < NC else {}

                    for s in range(TC):
                        t = c * TC + s
                        last = (t == t_steps - 1)
                        mb = KH * BS
                        m_cur = m_sb[:, s * mb:(s + 1) * mb].rearrange(
                            "p (j b) -> p j b", j=KH)
                        m_next = m_sb[:, (s + 1) * mb:(s + 2) * mb].rearrange(
                            "p (j b) -> p j b", j=KH)

                        xws = xw[:, :, s * BS:(s + 1) * BS]

                        def block_mms(pst, tiles, xw_lo, xw_hi):
                            """whh MMs for `tiles` + identity-MM folding in
                            the xw slice; single has_written group per bank"""
                            for ji, gt in enumerate(tiles):
                                for k in range(KH):
                                    nc.tensor.matmul(
                                        pst[:, ji, :],
                                        whh_sb[:, k * G + gt * 128:
                                               k * G + (gt + 1) * 128],
                                        h_in[:, k, :],
                                        start=(ji == 0 and k == 0),
                                        stop=False,
                                        skip_group_check=True,
                                    )
                            nc.tensor.matmul(
                                pst[:], ident_sb[:],
                                xws[:, xw_lo:xw_hi],
                                start=False, stop=True,
                                skip_group_check=True)

                        # g-candidates first: tanh(g) overlaps i/f MMs
                        block_mms(ps_g, [2 * KH + j for j in range(KH)],
                                  12, 16)
                        tng = spool.tile([128, KH, BS], f32, tag="tng")
                        nc.scalar.activation(tng[:], ps_g[:], Tanh)

                        block_mms(ps_if,
                                  [j for j in range(KH)]
                                  + [KH + j for j in range(KH)], 0, 8)
                        sif = spool.tile([128, 2 * KH, BS], f32, tag="sif")
                        nc.scalar.activation(sif[:], ps_if[:], Sigmoid)

                        block_mms(ps_o, [3 * KH + j for j in range(KH)],
                                  8, 12)
                        sfo = spool.tile([128, KH, BS], f32, tag="sfo")
                        nc.scalar.activation(sfo[:], ps_o[:], Sigmoid)

                        ig = spool.tile([128, KH, BS], f32, tag="ig")
                        nc.vector.tensor_mul(ig[:], sif[:, 0:KH], tng[:])
                        fc = spool.tile([128, KH, BS], f32, tag="fc")
                        nc.vector.tensor_mul(fc[:], sif[:, KH:2 * KH], cst[:])
                        cn = spool.tile([128, KH, BS], f32, tag="cn")
                        nc.vector.tensor_add(cn[:], ig[:], fc[:])
                        tc2 = spool.tile([128, KH, BS], f32, tag="tc2")
                        nc.scalar.activation(tc2[:], cn[:], Tanh)
                        if not last:
                            nc.gpsimd.tensor_mul(cst[:], cn[:], m_next)
                            sfom = spool.tile([128, KH, BS], f32, tag="sfom")
                            nc.gpsimd.tensor_mul(sfom[:], sfo[:], m_next)
                            nc.vector.tensor_mul(h_in[:], sfom[:], tc2[:])

                        if layer == 0:
                            t2y = spool.tile([128, KH, BS], f32, tag="t2y")
                            nc.gpsimd.tensor_mul(t2y[:], tc2[:], m_cur)
                            nc.gpsimd.tensor_mul(
                                ych[:, :, s, :], sfo[:], t2y[:])
                            nc.gpsimd.tensor_copy(
                                ychr[:, :, TC - 1 - s, :], ych[:, :, s, :])
                        else:
                            nc.gpsimd.tensor_mul(
                                ych[:, :, s, :], sfo[:], tc2[:])

                        hook = hooks.get(s)
                        if hook:
                            hook()

                    # chunk-granular stores (contiguous per partition)
                    if layer == 0:
                        nc.sync.dma_start(
                            y0[:, c].transpose([1, 0, 2, 3]), ych[:])
                        cr = NC - 1 - c
                        for k in range(KH):
                            nc.sync.dma_start(y0x[k * NC + cr], ychr[:, k])
                    else:
                        nc.sync.dma_start(
                            y1.ap()[:, c].transpose([1, 0, 2, 3]), ych[:])

            # hooks: spread chunk c+1's projection over chunk c's steps
            def _hooks(layer, c):
                cn_ = c + 1
                hooks = {}
                if layer == 0:
                    st = {}

                    def stage0():
                        st["rhs"] = stage_rhs_l0(cn_)
                        xw_tiles0[cn_] = new_xw_tile()
                    hooks[0] = stage0
                    slots = [4 + (i * 27) // 15 for i in range(GT)]

                    def mk(gidx):
                        def emit():
                            proj_tile(gidx, [(wih0_sb, st["rhs"])],
                                      xw_tiles0[cn_], bias_col=0)
                        return emit
                    for i, g in enumerate(range(GT)):
                        hooks[slots[i]] = mk(g)
                else:
                    st = {}

                    def stage1():
                        st["r"] = stage_rhs_l1(cn_)
                        xw_tiles1[cn_] = new_xw_tile()
                    hooks[0] = stage1
                    slots = [4 + (i * 27) // 15 for i in range(GT)]

                    def mk(gidx):
                        def emit():
                            orhs, prhs = st["r"]
                            proj_tile(gidx,
                                      [(wih1o_sb, orhs), (wih1p_sb, prhs)],
                                      xw_tiles1[cn_], bias_col=GT)
                        return emit
                    for i, g in enumerate(range(GT)):
                        hooks[slots[i]] = mk(g)
                return hooks

            # ---------- phase sequence ----------
            # prologue: proj chunk 0 of layer 0
            xw_tiles0 = [None] * NC
            xw_tiles1 = [None] * NC
            m_tiles = [None] * NC
            m_tiles[0] = new_mask_tile(0)
            rhs0 = stage_rhs_l0(0)
            xw_tiles0[0] = new_xw_tile()
            for g in range(GT):
                proj_tile(g, [(wih0_sb, rhs0)], xw_tiles0[0], bias_col=0)

            scan(whh0_sb, xw_tiles0, m_tiles, layer=0)

            # exchange
            nc.gpsimd.collective_compute(
                "AllGather", mybir.AluOpType.bypass,
                ins=[y0x.opt()], outs=[ag.opt()],
                replica_groups=[list(range(NCORES))],
            )

            # prologue: proj chunk 0 of layer 1
            m_tiles2 = [None] * NC
            m_tiles2[0] = new_mask_tile(0)
            orhs0, prhs0 = stage_rhs_l1(0)
            xw_tiles1[0] = new_xw_tile()
            for g in range(GT):
                proj_tile(g, [(wih1o_sb, orhs0), (wih1p_sb, prhs0)],
                          xw_tiles1[0], bias_col=GT)

            scan(whh1_sb, xw_tiles1, m_tiles2, layer=1)

    nc.compile()
    return nc


def _prep_inputs(x, lengths, weights, t_steps=T):
    """Build the 8 per-core input maps."""
    NC = t_steps // TC
    x = np.asarray(x, np.float32)
    lengths = np.asarray(lengths)
    active = (np.arange(T)[:, None] < lengths[None, :]).astype(np.float32)

    per_dir = {}
    for d, pre in ((0, "f"), (1, "b")):
        xs = x if d == 0 else x[::-1]
        am = active if d == 0 else active[::-1]
        xs = xs[:t_steps]
        am = am[:t_steps]
        xTd = np.ascontiguousarray(
            xs.transpose(2, 0, 1)).astype(bf16)  # [D, t, B]
        # mask rows: chunk c covers steps [c*TC, c*TC+TC] inclusive
        amp = np.vstack([am, np.ones((1, B), np.float32)])
        W_ih0 = np.asarray(weights[f"{pre}W_ih0"], np.float32)
        W_hh0 = np.asarray(weights[f"{pre}W_hh0"], np.float32)
        W_ih1 = np.asarray(weights[f"{pre}W_ih1"], np.float32)
        W_hh1 = np.asarray(weights[f"{pre}W_hh1"], np.float32)
        own = W_ih1[:, :H] if d == 0 else W_ih1[:, H:]
        par = W_ih1[:, H:] if d == 0 else W_ih1[:, :H]
        per_dir[d] = dict(
            xT=xTd, amp=amp,
            whh0T=np.ascontiguousarray(
                W_hh0.T.reshape(KH, 128, G)).astype(bf16),
            wih0T=np.ascontiguousarray(
                W_ih0.T.reshape(KH, 128, G)).astype(bf16),
            whh1T=np.ascontiguousarray(
                W_hh1.T.reshape(KH, 128, G)).astype(bf16),
            wih1oT=np.ascontiguousarray(
                own.T.reshape(KH, 128, G)).astype(bf16),
            wih1pT=np.ascontiguousarray(
                par.T.reshape(KH, 128, G)).astype(bf16),
            b0c=np.ascontiguousarray(
                np.asarray(weights[f"{pre}b0"], np.float32).reshape(GT, 128)),
            b1c=np.ascontiguousarray(
                np.asarray(weights[f"{pre}b1"], np.float32).reshape(GT, 128)),
        )

    in_maps = []
    for core in range(NCORES):
        d, s = core // 4, core % 4
        bsl = slice(s * BS, (s + 1) * BS)
        pd = per_dir[d]
        ams = pd["amp"][:, bsl]  # [t_steps+1, BS]
        maskrows = np.empty((NC, (TC + 1) * KH * BS), np.float32)
        for c in range(NC):
            blk = ams[c * TC:c * TC + TC + 1]            # [TC+1, BS]
            maskrows[c] = np.tile(blk, (1, KH)).reshape(-1)
        in_maps.append({
            "ident": np.eye(128, dtype=bf16),
            "xT": np.ascontiguousarray(pd["xT"][:, :, bsl]),
            "maskd": maskrows.astype(bf16),
            "whh0T": pd["whh0T"],
            "wih0T": pd["wih0T"],
            "whh1T": pd["whh1T"],
            "wih1oT": pd["wih1oT"],
            "wih1pT": pd["wih1pT"],
            "b0c": pd["b0c"],
            "b1c": pd["b1c"],
        })
    return in_maps


def _assemble(results, lengths, t_steps=T):
    NC = t_steps // TC
    lengths = np.asarray(lengths)
    active = (np.arange(t_steps)[:, None] < lengths[None, :])
    out = np.zeros((t_steps, B, 2 * H), np.float32)
    for core in range(NCORES):
        d, s = core // 4, core % 4
        arr = np.asarray(results[core]["y1"], dtype=bf16).astype(np.float32)
        # [KH, NC, 128, TC, BS] -> [t, b, j*128+p]
        blk = arr.transpose(1, 3, 4, 0, 2).reshape(t_steps, BS, H)
        if d == 1:
            blk = blk[::-1]
        out[:, s * BS:(s + 1) * BS, d * H:(d + 1) * H] = blk
    out *= active[:, :, None]
    return out


def kernel(x, lengths, fW_ih0, fW_hh0, fb0, bW_ih0, bW_hh0, bb0,
           fW_ih1, fW_hh1, fb1, bW_ih1, bW_hh1, bb1, _t_steps=T,
           _want_trace=False):
    weights = dict(fW_ih0=fW_ih0, fW_hh0=fW_hh0, fb0=fb0,
                   bW_ih0=bW_ih0, bW_hh0=bW_hh0, bb0=bb0,
                   fW_ih1=fW_ih1, fW_hh1=fW_hh1, fb1=fb1,
                   bW_ih1=bW_ih1, bW_hh1=bW_hh1, bb1=bb1)
    key = _t_steps
    if key not in _compiled:
        _compiled[key] = _build(_t_steps)
    nc = _compiled[key]
    in_maps = _prep_inputs(x, lengths, weights, _t_steps)
    res = bass_utils.run_bass_kernel_spmd(
        nc, in_maps, core_ids=list(range(NCORES)), trace=_want_trace)
    out = _assemble(res.results, lengths, _t_steps)
    if _want_trace:
        kernel.last_results = res
    return out
